# revision 3
# baseline (speedup 1.0000x reference)
"""Trainium2 Bass kernel for a 2-layer GAT (PyG GATConv semantics).

Strategy (8 NeuronCores, SPMD, 2 launches = 1 per GAT layer):
  - Destinations sharded across cores (6272 per core, incl. padding dsts).
  - Per layer, each core builds a full node-feature table in its DRAM:
      row(node n) = [h(n) | alpha_src(n) | alpha_dst(n)] in fp16,
    where h = x @ W, alpha_* = x @ (W @ att_*^T)  (computed by PE matmuls).
  - Edges are routed by destination on the host: for each 128-dst tile
    (destinations degree-sorted for tight padding), a rectangular slot grid
    [L, 128] holds edge source indices; slot (b, d) -> partition d, block b.
    dma_gather (SWDGE) fetches table rows for all slots of a tile in one
    instruction per table half (int16 indices limit a call to 32768 rows).
  - Attention: e = lrelu(alpha_src[src] + alpha_dst[dst]) comes straight from
    gathered columns; softmax per dst is a free-dim reduce since each
    partition holds exactly one destination's edge slots.
  - Weighted aggregation: scale gathered h by alpha (DVE), then accumulate
    slot-blocks into PSUM with identity matmuls (PE) -- slot (b, d) already
    sits on partition d, so lhsT=I sums blocks into the dst row.
  - Host work is routing/layout only: edge grouping, permutations,
    transposes, dtype casts, and the inter-layer relu'd feature relay.
"""

import sys

for _p in ("/opt/trn_rl_repo", "/root/.axon_site/_ro/trn_rl_repo"):
    if _p not in sys.path:
        sys.path.insert(0, _p)

import inspect
import textwrap
from contextlib import ExitStack

import numpy as np

import os

import concourse.bass as _bassmod
import concourse.tile as tile
from concourse import bacc, mybir
from concourse.bass_utils import run_bass_kernel_spmd

# set GAT_TRACE=1 to profile each launch; exec times land in LAST_EXEC_NS
LAST_EXEC_NS = []
LAST_RES = []  # full BassKernelResults when tracing (trace paths, profile json)

# scheduling / layout knobs (A/B-tested via TimelineSim)
CFG = {
    "gpool_bufs": 3,
    "epool_bufs": 3,
    "bpsum_bufs": 4,
    "opsum_bufs": 2,
    "build_bufs": 3,
    "single_packet": False,
    "queue_spread": 0,  # 0: all gathers on q0; 1: A on q0 / B on q1
    "skip_build": False,
    "xchunk": 16,
    "dma_scratch": 16384,  # SWDGE ring: descs = this/16
    "wchunk": 8,  # node tiles per table-write DMA
    "act_copies": True,  # alternate build psum->sbuf copies DVE/ACT
    "build_b_first": False,  # build tableB before tableA; B-gathers first
    "skip_attn": False,  # debug: skip e/exp/alpha chain
    "skip_scale": False,  # debug: feed G directly to PE (no alpha scaling)
    "skip_pe": False,  # debug: skip accumulation matmuls
}

f32 = mybir.dt.float32
f16 = mybir.dt.float16
i16 = mybir.dt.int16

P = 128
NEG_SLOPE = 0.2
NEG_BIG = -60000.0  # alpha_src of pad rows: exp(lrelu(-60000)) == 0
HALF = 32768  # dma_gather int16 index window (rows per table view)


def _patch_dma_gather():
    """Relax the %256 elem-size assert (the ucode only needs it for transpose)."""
    if getattr(_bassmod.BassGpSimd.dma_gather, "_gat_patched", False):
        return
    src = inspect.getsource(_bassmod.BassGpSimd.dma_gather)
    old = """        assert (
            elem_size_bytes > 0 and elem_size_bytes % 256 == 0
        )  # transpose restriction"""
    new = """        assert elem_size_bytes > 0
        if transpose:
            assert elem_size_bytes % 256 == 0"""
    assert old in src, "dma_gather source changed; patch needs updating"
    src = textwrap.dedent(src.replace(old, new))
    ns = dict(_bassmod.__dict__)
    exec(compile(src, "<dma_gather_patched>", "exec"), ns)
    ns["dma_gather"]._gat_patched = True
    _bassmod.BassGpSimd.dma_gather = ns["dma_gather"]


# ---------------------------------------------------------------- host routing


class EdgePlan:
    """Destination-sharded edge routing shared by both layers."""

    def __init__(self, src, dst, n_nodes, n_cores, half_n):
        self.n_nodes = n_nodes
        self.n_cores = n_cores
        self.half_n = half_n  # nodes < half_n gather via table view A
        self.half_base = half_n + 1  # B table view starts at this row
        assert self.half_base <= HALF and n_nodes - half_n < HALF
        self.dpc = int(np.ceil(n_nodes / n_cores / P)) * P  # dsts per core
        self.n_tiles = self.dpc // P

        src = np.asarray(src, dtype=np.int64)
        dst = np.asarray(dst, dtype=np.int64)

        # per-core CSR by local dst
        self.core_edges = []  # (perm, degA, degB, srcs_sorted, offsets)
        for c in range(n_cores):
            lo, hi = c * self.dpc, (c + 1) * self.dpc
            m = (dst >= lo) & (dst < hi)
            d_loc = (dst[m] - lo).astype(np.int32)
            s = src[m].astype(np.int32)
            # order edges per dst, A-half (src < half_n) before B-half
            is_b = (s >= half_n).astype(np.int32)
            order = np.lexsort((is_b, d_loc))
            d_loc, s, is_b = d_loc[order], s[order], is_b[order]
            degA = np.bincount(d_loc[is_b == 0], minlength=self.dpc)
            degB = np.bincount(d_loc[is_b == 1], minlength=self.dpc)
            deg = degA + degB
            offs = np.zeros(self.dpc + 1, np.int64)
            np.cumsum(deg, out=offs[1:])
            # sort dsts: degA desc, then snake-ordered degB for tight B grids
            snake = np.where(degA % 2 == 0, degB, -degB)
            perm = np.lexsort((snake, -degA)).astype(np.int32)
            self.core_edges.append((perm, degA, degB, s, offs))

        # global (SPMD-uniform) per-tile slot counts
        self.LA = np.zeros(self.n_tiles, np.int64)
        self.LB = np.zeros(self.n_tiles, np.int64)
        for perm, degA, degB, _, _ in self.core_edges:
            pa = degA[perm].reshape(self.n_tiles, P)
            pb = degB[perm].reshape(self.n_tiles, P)
            np.maximum(self.LA, pa.max(axis=1), out=self.LA)
            np.maximum(self.LB, pb.max(axis=1), out=self.LB)
        self.group = 2  # tiles per gather call
        self.blocks = self.LA + self.LB
        # idx columns per gather call: L*128 indices -> L*8 int16 columns
        self.colsA = self.LA * 8
        self.colsB = self.LB * 8
        # plus the upfront dst segment: 2 blocks (A, B) per tile
        self.W = int((self.colsA + self.colsB).sum()) + 2 * self.n_tiles * 8

    def n_table_rows(self, n_node_tiles):
        # row 0 = PAD_A, rows 1..N = nodes, build writes 1..1+128*ntiles,
        # PAD_B right after the build region.
        padb = 1 + n_node_tiles * P
        return padb + 1, padb

    @staticmethod
    def _wrap16(flat):
        w = np.zeros((16, flat.size // 16), np.int16)
        ar = np.arange(flat.size)
        w[ar % 16, ar // 16] = flat
        return np.tile(w, (8, 1))

    def build_idx(self, core, padb_row):
        """int16 idx array [128, W]: per gather-group [A-srcs | B-srcs] grids
        (16-wrapped, 8-replicated), then the dst-row segment (2 blocks/tile)."""
        perm, degA, degB, srcs, offs = self.core_edges[core]
        half = self.half_n
        hb = self.half_base
        grp = self.group
        segs = []
        for g0 in range(0, self.n_tiles, grp):
            gn = min(grp, self.n_tiles - g0)
            for half_sel in (0, 1):
                parts = []
                for k in range(g0, g0 + gn):
                    dsts = perm[k * P : (k + 1) * P]
                    if half_sel == 0:
                        L = int(self.LA[k])
                        grid = np.full((L, P), 0, np.int32)
                        for j, d in enumerate(dsts):
                            o, da = offs[d], degA[d]
                            if da:
                                grid[:da, j] = srcs[o : o + da] + 1
                    else:
                        L = int(self.LB[k])
                        grid = np.full((L, P), padb_row - hb, np.int32)
                        for j, d in enumerate(dsts):
                            o, da, db = offs[d], degA[d], degB[d]
                            if db:
                                grid[:db, j] = srcs[o + da : o + da + db] + 1 - hb
                    parts.append(grid.reshape(-1))
                segs.append(self._wrap16(np.concatenate(parts)))
        # dst-row segment: for each tile, one A-block and one B-block
        dparts = []
        for k in range(self.n_tiles):
            dsts = perm[k * P : (k + 1) * P]
            node = core * self.dpc + dsts
            valid = node < self.n_nodes
            ga = np.where(valid & (node < half), node + 1, 0)
            gb = np.where(valid & (node >= half), node + 1 - hb, padb_row - hb)
            dparts.append((ga.astype(np.int32), gb.astype(np.int32)))
        segs.append(self._wrap16(np.concatenate([a for a, _ in dparts])))
        segs.append(self._wrap16(np.concatenate([b for _, b in dparts])))
        return np.concatenate(segs, axis=1)

    def unpermute(self, core_outs, fout):
        """core_outs: list of [dpc, fout] arrays (permuted dst order)."""
        full = np.zeros((self.n_nodes, fout), np.float32)
        for c, arr in enumerate(core_outs):
            perm = self.core_edges[c][0]
            node = c * self.dpc + perm
            m = node < self.n_nodes
            full[node[m]] = arr[m]
        return full


# ------------------------------------------------------------- device program


def build_layer_program(plan: EdgePlan, fin, n_heads, ch, relu, n_cores):
    """One GAT layer: table build + edge aggregation. Returns compiled Bacc."""
    _patch_dma_gather()
    outf = n_heads * ch
    rowv = outf + 2 * n_heads  # [h | alpha_src | alpha_dst]
    pitch = 1 << int(np.ceil(np.log2(rowv)))  # fp16 row pitch (values)
    assert pitch * 2 % 256 == 0
    n_node_tiles = int(np.ceil(plan.n_nodes / P))
    nodes_pad = n_node_tiles * P
    n_rows, padb_row = plan.n_table_rows(n_node_tiles)

    nc = bacc.Bacc(
        "TRN2",
        target_bir_lowering=False,
        debug=False,
        num_devices=n_cores,
        dynamic_dma_scratch_size=CFG["dma_scratch"],
    )
    xT = nc.dram_tensor("xT", [P, nodes_pad], f16, kind="ExternalInput").ap()
    wext = nc.dram_tensor("wext", [P, rowv], f16, kind="ExternalInput").ap()
    bias = nc.dram_tensor("bias", [P, outf], f32, kind="ExternalInput").ap()
    ident_in = nc.dram_tensor("ident", [P, P], f16, kind="ExternalInput").ap()
    idx_in = nc.dram_tensor("idx", [P, plan.W], i16, kind="ExternalInput").ap()
    out = nc.dram_tensor("out", [plan.dpc, outf], f32, kind="ExternalOutput").ap()
    hb = plan.half_base
    tableA = nc.dram_tensor("tableA", [hb, pitch], f16)
    tableB = nc.dram_tensor("tableB", [n_rows - hb, pitch], f16)

    XCH = CFG["xchunk"]  # node tiles per xT load chunk

    with tile.TileContext(nc) as tc, ExitStack() as ctx:
        const = ctx.enter_context(tc.tile_pool(name="const", bufs=1))
        build = ctx.enter_context(tc.tile_pool(name="build", bufs=CFG["build_bufs"]))
        bpsum = ctx.enter_context(
            tc.tile_pool(name="bpsum", bufs=CFG["bpsum_bufs"], space="PSUM")
        )
        gpool = ctx.enter_context(tc.tile_pool(name="gpool", bufs=CFG["gpool_bufs"]))
        epool = ctx.enter_context(tc.tile_pool(name="epool", bufs=CFG["epool_bufs"]))
        opsum = ctx.enter_context(
            tc.tile_pool(name="opsum", bufs=CFG["opsum_bufs"], space="PSUM")
        )

        ident = const.tile([P, P], f16)
        nc.sync.dma_start(out=ident[:], in_=ident_in[:])
        wext_sb = const.tile([P, rowv], f16)
        nc.sync.dma_start(out=wext_sb[:], in_=wext[:])
        bias_sb = const.tile([P, outf], f32)
        nc.sync.dma_start(out=bias_sb[:], in_=bias[:])
        idx_sb = const.tile([P, plan.W], i16)
        nc.sync.dma_start(out=idx_sb[:], in_=idx_in[:])

        # ---- table build: rows 1 .. 1+128*n_node_tiles
        WCH = CFG["wchunk"]
        assert XCH % WCH == 0
        chunk_order = list(range(0, n_node_tiles, XCH))
        if CFG["build_b_first"]:
            bsplit = (hb - 1) // P // XCH  # first chunk touching tableB
            chunk_order = chunk_order[bsplit:] + chunk_order[:bsplit]
        for c0 in ([] if CFG["skip_build"] else chunk_order):
            cn = min(XCH, n_node_tiles - c0)
            xchunk = build.tile([P, XCH * P], f16, tag="xchunk")
            nc.sync.dma_start(
                out=xchunk[:, : cn * P], in_=xT[:, c0 * P : (c0 + cn) * P]
            )
            for w0 in range(0, cn, WCH):
                wn = min(WCH, cn - w0)
                row_sb = build.tile([P, WCH * rowv], f16, tag="rowsb")
                for t in range(w0, w0 + wn):
                    ps = bpsum.tile([P, rowv], f32, space="PSUM", tag="bps")
                    nc.tensor.matmul(
                        out=ps[:],
                        lhsT=xchunk[:, t * P : (t + 1) * P],
                        rhs=wext_sb[:],
                        start=True,
                        stop=True,
                    )
                    dst_sl = row_sb[:, (t - w0) * rowv : (t - w0 + 1) * rowv]
                    if CFG["act_copies"] and t % 2 == 1:
                        nc.scalar.copy(dst_sl, ps[:])
                    else:
                        nc.vector.tensor_copy(out=dst_sl, in_=ps[:])
                r0 = 1 + (c0 + w0) * P
                r1 = r0 + wn * P
                # write to tableA/tableB (split if the chunk crosses half_base)
                for lo, hi, tab, base in (
                    (r0, min(r1, hb), tableA, 0),
                    (max(r0, hb), r1, tableB, hb),
                ):
                    if lo >= hi:
                        continue
                    t0 = (lo - r0) // P
                    tn = (hi - lo + P - 1) // P
                    if (lo - r0) % P == 0 and (hi - lo) % P == 0:
                        nc.sync.dma_start(
                            out=tab[lo - base : hi - base, :rowv].rearrange(
                                "(t p) v -> p t v", t=tn
                            ),
                            in_=row_sb[:, t0 * rowv : (t0 + tn) * rowv].rearrange(
                                "p (t v) -> p t v", t=tn
                            ),
                        )
                    else:
                        rr = lo
                        while rr < hi:
                            tt = (rr - r0) // P
                            po = (rr - r0) % P
                            rn = min(P - po, hi - rr)
                            nc.sync.dma_start(
                                out=tab[rr - base : rr - base + rn, :rowv],
                                in_=row_sb[po : po + rn, tt * rowv : (tt + 1) * rowv],
                            )
                            rr += rn

        # ---- pad rows (row 0 = PAD_A, row padb_row = PAD_B)
        padrow = build.tile([1, pitch], f16, tag="padrow")
        nc.vector.memset(padrow[:], 0.0)
        nc.vector.memset(padrow[:, outf : outf + n_heads], NEG_BIG)
        nc.sync.dma_start(out=tableA[0:1, :rowv], in_=padrow[:, :rowv])
        nc.sync.dma_start(
            out=tableB[padb_row - hb : padb_row - hb + 1, :rowv],
            in_=padrow[:, :rowv],
        )

        # ---- edge phase
        # Upfront: gather every tile's dst rows (alpha_dst source), one call
        # per table half. Then per gather-group (plan.group tiles), one call
        # per half fetches all src slot-blocks; per tile the compute is a
        # handful of fused DVE/ACT ops + identity-matmul PSUM accumulation.
        n_t = plan.n_tiles
        dcol = int((plan.colsA + plan.colsB).sum())
        DST = const.tile([P, 2 * n_t * rowv], f16)
        DST3 = DST[:].rearrange("p (b v) -> p b v", b=2 * n_t, v=rowv)
        dst_calls = [
            (DST3[:, :n_t, :], tableA, idx_sb[:, dcol : dcol + n_t * 8]),
            (
                DST3[:, n_t :, :],
                tableB,
                idx_sb[:, dcol + n_t * 8 : dcol + 2 * n_t * 8],
            ),
        ]
        if CFG["build_b_first"]:
            dst_calls.reverse()
        for d_out, d_tab, d_idx in dst_calls:
            nc.gpsimd.dma_gather(
                out_ap=d_out,
                in_ap=d_tab[:, :rowv],
                idxs_ap=d_idx,
                num_idxs=n_t * P,
                num_idxs_reg=n_t * P,
                elem_size=rowv,
                elem_step=pitch,
                single_packet=CFG["single_packet"],
            )

        # virtual heads: single-head layers pair adjacent channels so the
        # alpha scale still runs in the DVE packed (2x/4x) mode
        hv = n_heads if n_heads > 1 else 2
        chv = outf // hv

        grp = plan.group
        col = 0
        for g0 in range(0, n_t, grp):
            gn = min(grp, n_t - g0)
            gLA = [int(plan.LA[k]) for k in range(g0, g0 + gn)]
            gLB = [int(plan.LB[k]) for k in range(g0, g0 + gn)]
            nA, nB = sum(gLA), sum(gLB)
            BT = nA + nB
            G = gpool.tile([P, BT * rowv], f16, tag="G")
            G3 = G[:].rearrange("p (b v) -> p b v", b=BT, v=rowv)
            gather_calls = [
                (G3[:, :nA, :], tableA, idx_sb[:, col : col + nA * 8], nA),
                (
                    G3[:, nA:, :],
                    tableB,
                    idx_sb[:, col + nA * 8 : col + (nA + nB) * 8],
                    nB,
                ),
            ]
            if CFG["build_b_first"]:
                gather_calls.reverse()
            for g_out, g_tab, g_idx, g_n in gather_calls:
                nc.gpsimd.dma_gather(
                    out_ap=g_out,
                    in_ap=g_tab[:, :rowv],
                    idxs_ap=g_idx,
                    num_idxs=g_n * P,
                    num_idxs_reg=g_n * P,
                    elem_size=rowv,
                    elem_step=pitch,
                    single_packet=CFG["single_packet"],
                )
            col += (nA + nB) * 8

            offA, offB = 0, nA
            for ki in range(gn):
                k = g0 + ki
                LA, LB = gLA[ki], gLB[ki]
                LT = LA + LB
                # this tile's src blocks: [offA, offA+LA) and [offB, offB+LB)
                ranges = [(offA, 0, LA), (offB, LA, LB)]
                offA += LA
                offB += LB

                alpha = epool.tile([P, LT * n_heads], f16, tag="alpha")
                alpha3 = alpha[:].rearrange("p (l h) -> p l h", l=LT, h=n_heads)
                if CFG["skip_attn"]:
                    nc.vector.memset(alpha[:], 1.0)
                else:
                    ad = epool.tile([P, n_heads], f16, tag="ad")
                    nc.vector.tensor_tensor(
                        out=ad[:],
                        in0=DST3[:, k, outf + n_heads : outf + 2 * n_heads],
                        in1=DST3[:, n_t + k, outf + n_heads : outf + 2 * n_heads],
                        op=mybir.AluOpType.add,
                    )
                    E = epool.tile([P, LT * n_heads], f16, tag="E")
                    E3 = E[:].rearrange("p (l h) -> p l h", l=LT, h=n_heads)
                    for gbase, lbase, n in ranges:
                        if n:
                            nc.vector.tensor_tensor(
                                out=E3[:, lbase : lbase + n, :],
                                in0=G3[:, gbase : gbase + n, outf : outf + n_heads],
                                in1=ad[:].unsqueeze(1).to_broadcast([P, n, n_heads]),
                                op=mybir.AluOpType.add,
                            )
                    T = epool.tile([P, LT * n_heads], f16, tag="T")
                    nc.vector.scalar_tensor_tensor(
                        out=T[:],
                        in0=E[:],
                        scalar=NEG_SLOPE,
                        in1=E[:],
                        op0=mybir.AluOpType.mult,
                        op1=mybir.AluOpType.max,
                    )
                    W = epool.tile([P, LT * n_heads], f32, tag="W")
                    nc.scalar.activation(W[:], T[:], mybir.ActivationFunctionType.Exp)
                    W3 = W[:].rearrange("p (l h) -> p l h", l=LT, h=n_heads)
                    den = epool.tile([P, n_heads], f32, tag="den")
                    nc.vector.tensor_reduce(
                        out=den[:],
                        in_=W3.transpose([0, 2, 1]),
                        axis=mybir.AxisListType.X,
                        op=mybir.AluOpType.add,
                    )
                    nc.vector.tensor_scalar_add(den[:], den[:], 1e-16)
                    rec = epool.tile([P, n_heads], f32, tag="rec")
                    nc.vector.reciprocal(rec[:], den[:])
                    nc.vector.tensor_tensor(
                        out=alpha3,
                        in0=W3,
                        in1=rec[:].unsqueeze(1).to_broadcast([P, LT, n_heads]),
                        op=mybir.AluOpType.mult,
                    )

                # alpha in (l, hv) layout for the packed-mode scale
                if n_heads == 1:
                    adup = epool.tile([P, LT * hv], f16, tag="adup")
                    nc.vector.tensor_copy(
                        out=adup[:].rearrange("p (l j) -> p l j", l=LT, j=hv),
                        in_=alpha3.to_broadcast([P, LT, hv]),
                    )
                    alpha_v = adup[:].rearrange("p (l j) -> p l j", l=LT, j=hv)
                else:
                    alpha_v = alpha3

                ps = opsum.tile([P, outf], f32, space="PSUM", tag="ops")
                if CFG["skip_pe"]:
                    o_sb = epool.tile([P, outf], f32, tag="osb")
                    nc.vector.tensor_copy(out=o_sb[:], in_=DST3[:, k, :outf])
                    nc.sync.dma_start(out=out[k * P : (k + 1) * P, :], in_=o_sb[:])
                    continue
                if CFG["skip_scale"]:
                    j = 0
                    for gbase, lbase, n in ranges:
                        for b in range(n):
                            nc.tensor.matmul(
                                out=ps[:],
                                lhsT=ident[:],
                                rhs=G3[:, gbase + b, :outf],
                                start=(j == 0),
                                stop=(j == LT - 1),
                            )
                            j += 1
                else:
                    wG = epool.tile([P, LT * outf], f16, tag="wG")
                    wG4 = wG[:].rearrange(
                        "p (l c h) -> p l c h", l=LT, c=chv, h=hv
                    )
                    for gbase, lbase, n in ranges:
                        if n:
                            nc.vector.tensor_tensor(
                                out=wG4[:, lbase : lbase + n],
                                in0=G3[:, gbase : gbase + n, :outf].rearrange(
                                    "p l (c h) -> p l c h", c=chv, h=hv
                                ),
                                in1=alpha_v[:, lbase : lbase + n, :]
                                .unsqueeze(2)
                                .to_broadcast([P, n, chv, hv]),
                                op=mybir.AluOpType.mult,
                            )
                    wG3 = wG[:].rearrange("p (l f) -> p l f", l=LT, f=outf)
                    for j in range(LT):
                        nc.tensor.matmul(
                            out=ps[:],
                            lhsT=ident[:],
                            rhs=wG3[:, j, :],
                            start=(j == 0),
                            stop=(j == LT - 1),
                        )

                o_sb = epool.tile([P, outf], f32, tag="osb")
                nc.vector.tensor_tensor(
                    out=o_sb[:], in0=ps[:], in1=bias_sb[:], op=mybir.AluOpType.add
                )
                if relu:
                    nc.vector.tensor_scalar_max(o_sb[:], o_sb[:], 0.0)
                nc.sync.dma_start(out=out[k * P : (k + 1) * P, :], in_=o_sb[:])

    nc.compile()
    return nc, padb_row, nodes_pad


# ------------------------------------------------------------------ execution


def _prep_wext(W, att_src, att_dst):
    """[fin, outf + 2H] fp16: [W (c-major cols) | W @ att_src^T | W @ att_dst^T].

    Feature columns are emitted c-major ((c, h), h fastest) to enable the DVE
    packed mode on-device; outputs are de-interleaved on the host."""
    H, C = att_src.shape
    fin = W.shape[0]
    Wr = W.reshape(fin, H, C)
    a_s = np.einsum("fhc,hc->fh", Wr, att_src)
    a_d = np.einsum("fhc,hc->fh", Wr, att_dst)
    Wi = Wr.transpose(0, 2, 1).reshape(fin, H * C)  # (c, h) column order
    return np.concatenate([Wi, a_s, a_d], axis=1).astype(np.float16)


def _interleave_cols(v, H, C):
    return np.asarray(v, np.float32).reshape(H, C).T.reshape(H * C)


def _deinterleave(arr, H, C):
    """[n, (c h)] -> [n, (h c)]"""
    n = arr.shape[0]
    return arr.reshape(n, C, H).transpose(0, 2, 1).reshape(n, H * C)


def _xT_f16(x, nodes_pad):
    n = x.shape[0]
    xt = np.zeros((x.shape[1], nodes_pad), np.float16)
    xt[:, :n] = np.asarray(x, np.float32).T.astype(np.float16)
    return xt


def run_layer(plan, nc_bundle, x, W, att_src, att_dst, b, relu, n_cores, idx_arrs):
    nc, padb_row, nodes_pad = nc_bundle
    H, C = att_src.shape
    outf = H * C
    wext = _prep_wext(np.asarray(W, np.float32), np.asarray(att_src, np.float32),
                      np.asarray(att_dst, np.float32))
    xt = _xT_f16(x, nodes_pad)
    bias = np.broadcast_to(_interleave_cols(b, H, C), (P, outf)).copy()
    ident = np.eye(P, dtype=np.float16)
    in_maps = [
        {"xT": xt, "wext": wext, "bias": bias, "ident": ident, "idx": idx_arrs[c]}
        for c in range(n_cores)
    ]
    trace = os.environ.get("GAT_TRACE", "") == "1"
    res = run_bass_kernel_spmd(nc, in_maps, list(range(n_cores)), trace=trace)
    if trace:
        LAST_EXEC_NS.append(res.exec_time_ns)
        LAST_RES.append(res)
    outs = [res.results[c]["out"] for c in range(n_cores)]
    return _deinterleave(plan.unpermute(outs, outf), H, C)


def gat_forward(x, edge_index, params, n_cores=8, half_n=HALF - 1):
    """params: (W1, as1, ad1, b1, W2, as2, ad2, b2). Returns [N, F2] fp32."""
    x = np.asarray(x, np.float32)
    n = x.shape[0]
    ei = np.asarray(edge_index)
    loop = np.arange(n, dtype=ei.dtype)
    src = np.concatenate([ei[0], loop])
    dst = np.concatenate([ei[1], loop])

    plan = EdgePlan(src, dst, n, n_cores, half_n)
    W1, as1, ad1, b1, W2, as2, ad2, b2 = params

    bundle1 = build_layer_program(plan, x.shape[1], as1.shape[0], as1.shape[1],
                                  relu=True, n_cores=n_cores)
    idx_arrs = [plan.build_idx(c, bundle1[1]) for c in range(n_cores)]
    h = run_layer(plan, bundle1, x, W1, as1, ad1, b1, True, n_cores, idx_arrs)

    bundle2 = build_layer_program(plan, h.shape[1], as2.shape[0], as2.shape[1],
                                  relu=False, n_cores=n_cores)
    if bundle2[1] != bundle1[1]:
        idx_arrs = [plan.build_idx(c, bundle2[1]) for c in range(n_cores)]
    out = run_layer(plan, bundle2, h, W2, as2, ad2, b2, False, n_cores, idx_arrs)
    return out


def kernel(x, edge_index, W1, att_src1, att_dst1, b1, W2, att_src2, att_dst2, b2):
    params = tuple(
        np.asarray(a, np.float32)
        for a in (W1, att_src1, att_dst1, b1, W2, att_src2, att_dst2, b2)
    )
    return gat_forward(x, edge_index, params).astype(np.float32)

